# revision 1
# baseline (speedup 1.0000x reference)
"""Trainium2 Bass kernel for EntropySamplLoss, v8 (bf16 inputs, ACT+DVE).

Reference semantics (per image b):
  acts [N, P=320] viewed as [N, S=4, C=8, K=10] prototype groups
  ent[n, s, c] = normalized softmax entropy over the K protos of group (s, c)
  loss = mean over present (b, s, c) of (sum of ent over pixels with label c)
         / (count of pixels with label c)

Device kernel (data-parallel, one image per NeuronCore).  Per-pixel-group
entropy ent = logZ - U/Z with Z = sum_k e^x, U = sum_k x e^x.

v7 (475 us) proved DVE and GPSIMD serialize on SBUF ports: DVE tensor ops
overlapping a GPSIMD mult run 5-8x slow, so GPSIMD-seconds cost
DVE-seconds and the GPSIMD offload (5.0 us/chunk) loses to doing the same
multiply on DVE (1.5 us at bf16 2x).  v8 drops GPSIMD and instead:

  - uploads acts as bf16 (host-side RNE cast; per-element noise ~0.2% is
    random and averages out over 2M pixel-groups).  Halves HBM traffic
    (DMA floor 64 x 0.66 MB at ~370 GB/s ~ 114 us), halves a-tile SBUF
    reads, and makes x available in bf16 so the x*E multiply hits DVE 2x.
  - host stores acts k-major per chunk ((k, j, g)) so every on-chip
    operand is contiguous/packed -> all DVE tree levels at bf16 2x
  - all chunks: E = exp(x) -> bf16           (ACT, 2.41 us)
  - A-chunks (~22): SY = silu(x - 12) ~ (x-12)e^(x-12)  (ACT, 2.42 us;
    e^12*SY + 12*Z recovers U -- silu identity)
  - C-chunks (rest): XE = x * E              (DVE bf16 2x mult, 1.49 us)
  - combined 4-level DVE tree over chunk PAIRS (halves instr overhead)
    sums E and SY/XE planes -> Z, U-part     (DVE, 2.71 us/chunk)
  - logZ = ln(Z) and rz = exp(-logZ) batched per 8-chunk group on ACT
    (both in the pinned exp+ln table set; silu needs its own table, so
    A-chunks are packed into 4 of the 8 groups -> 8 table loads)
  - mx = (SY/XE sum) * rz: one DVE bf16 2x mult per group
  - PE accumulates mask^T @ [logZ | 1 | m-indicator] and mask^T @ mx,
    with separate PSUM accumulators for A-chunk and C-chunk mx so the
    host applies the e^12/+12 silu-identity correction to A rows only.

Measured: 277.0 us HW exec (v6 baseline 389.5 us, 1.41x), rel err 2.4e-04.
"""

import sys

if "/opt/trn_rl_repo" not in sys.path:
    sys.path.insert(0, "/opt/trn_rl_repo")

from contextlib import ExitStack

import numpy as np

import concourse.bacc as bacc
import concourse.bass as bass
import concourse.tile as tile
from concourse import mybir
from concourse.bass_utils import run_bass_kernel_spmd

# Problem shape (hardcoded per spec)
B, N, PP = 8, 65536, 320
S, C, K = 4, 8, 10
NCORES = 8

PX_PER_PART = 8                        # pixels per partition ("j" slots)
PART = 128
PX_PER_CHUNK = PART * PX_PER_PART      # 1024
NCHUNK = N // PX_PER_CHUNK             # 64
FREE = PX_PER_PART * PP                # 2560
G = S * C                              # 32 groups per pixel
GF = PX_PER_PART * G                   # 256 group slots per partition
EW = G + 2                             # 34: ent cols + ones col + m col
PH = 8                                 # chunks per group (8 uniform groups)
# Uniform interleave: every group pays the same 2 ACT table loads anyway
# (the scheduler interleaves groups), so spread A-chunks evenly to keep
# per-group ACT and DVE loads balanced and avoid pipeline phase stalls.
A_GROUPS = {0: 3, 1: 3, 2: 3, 3: 2, 4: 2, 5: 2, 6: 2, 7: 2}
MSHIFT = 12.0

_CACHE = {}


def _patch_act_tables():
    """Make the combined exp+ln table set the only candidate for Exp/Ln so
    the table-load placement pass doesn't thrash between per-function sets."""
    import concourse.hw_specs as hw_specs

    tabs = hw_specs.get_activation_tables("gen3")
    E = mybir.ActivationFunctionType.Exp
    L = mybir.ActivationFunctionType.Ln
    for name, funcs in tabs.items():
        if name != "natural_log_exp_and_others":
            funcs.discard(E)
            funcs.discard(L)


def _groups():
    """Per group: (chunk list, na).  Variant layout within a group is
    [C]*(len-na) + [A]*na -- A-chunks (silu on ACT) at the TAIL so DVE can
    start the leading C-chunks' multiplies and trees right after their
    exps instead of waiting for the ACT silu burst.  C-chunks use the
    stats2b accumulator, A-chunks stats2a."""
    out = []
    for gi, g0 in enumerate(range(0, NCHUNK, PH)):
        group = list(range(g0, min(g0 + PH, NCHUNK)))
        na = A_GROUPS.get(gi, 0)
        assert na <= len(group)
        out.append((group, na))
    return out


def _variant_chunks():
    a_chunks, c_chunks = [], []
    for group, na in _groups():
        c_chunks.extend(group[: len(group) - na])
        a_chunks.extend(group[len(group) - na :])
    return a_chunks, c_chunks


def _build():
    if "nc" in _CACHE:
        return _CACHE["nc"]

    _patch_act_tables()
    f32 = mybir.dt.float32
    bf16 = mybir.dt.bfloat16
    nc = bacc.Bacc("TRN2", target_bir_lowering=False, debug=False, num_devices=NCORES)

    acts = nc.dram_tensor(
        "acts", [NCHUNK, PART, FREE], bf16, kind="ExternalInput"
    ).ap()
    labels = nc.dram_tensor(
        "labels", [PART, NCHUNK * PX_PER_PART], f32, kind="ExternalInput"
    ).ap()
    consts = nc.dram_tensor("consts", [C + 1], f32, kind="ExternalInput")
    stats_out = nc.dram_tensor(
        "stats", [PX_PER_PART * C, PX_PER_PART * EW], f32, kind="ExternalOutput"
    ).ap()
    stats2a_out = nc.dram_tensor(
        "stats2a", [PX_PER_PART * C, PX_PER_PART * G], f32, kind="ExternalOutput"
    ).ap()
    stats2b_out = nc.dram_tensor(
        "stats2b", [PX_PER_PART * C, PX_PER_PART * G], f32, kind="ExternalOutput"
    ).ap()

    a_chunks, c_chunks = _variant_chunks()
    first_a, last_a = a_chunks[0], a_chunks[-1]
    first_c, last_c = c_chunks[0], c_chunks[-1]

    with tile.TileContext(nc) as tc:
        with ExitStack() as ctx:
            singles = ctx.enter_context(tc.tile_pool(name="singles", bufs=1))
            apool = ctx.enter_context(tc.tile_pool(name="apool", bufs=8))
            expool = ctx.enter_context(tc.tile_pool(name="expool", bufs=3))
            tree = ctx.enter_context(tc.tile_pool(name="tree", bufs=2))
            zpool = ctx.enter_context(tc.tile_pool(name="zpool", bufs=3))
            lzpool = ctx.enter_context(tc.tile_pool(name="lzpool", bufs=3))
            rzpool = ctx.enter_context(tc.tile_pool(name="rzpool", bufs=3))
            mxpool = ctx.enter_context(tc.tile_pool(name="mxpool", bufs=3))
            mkpool = ctx.enter_context(tc.tile_pool(name="mkpool", bufs=3))
            psum = ctx.enter_context(tc.tile_pool(name="psum", bufs=2, space="PSUM"))

            # constants: [1..8, 1.0] broadcast to all partitions
            cvec = singles.tile([PART, C + 1], f32)
            consts_b = bass.AP(tensor=consts, offset=0, ap=[[0, PART], [1, C + 1]])
            nc.sync.dma_start(out=cvec[:], in_=consts_b)
            # copy on DVE, not ACT: a scalar.copy here costs an extra ACT
            # table cycle that delays the first exp by ~2.5us
            iota_ps = psum.tile([PART, C], f32)
            nc.vector.tensor_copy(out=iota_ps[:], in_=cvec[:, 0:C])
            mvec = singles.tile([PART, 1], f32)
            nc.vector.memset(mvec[:], -MSHIFT)

            # all labels resident: [128 part, 64 chunk, 8 j] (2 KB/partition)
            lab_sb = singles.tile([PART, NCHUNK, PX_PER_PART], f32)
            nc.sync.dma_start(
                out=lab_sb[:].rearrange("p c j -> p (c j)"), in_=labels
            )

            stats_ps = psum.tile([PX_PER_PART * C, PX_PER_PART * EW], f32)
            stats2a_ps = psum.tile([PX_PER_PART * C, PX_PER_PART * G], f32)
            stats2b_ps = psum.tile([PX_PER_PART * C, PX_PER_PART * G], f32)

            for group, na in _groups():
                gn = len(group)
                g0 = group[0]
                a_t, ex_t = {}, {}
                # exsy PAIR tiles: [128, 2 (chunk), 2 (m), K, GF] -- trees
                # run per pair; exp/silu/mult stay per-chunk (finer-grained
                # pipelining measured faster than pair-batched transforms)
                for pi in range(gn // 2):
                    e_pair = expool.tile([PART, 2, 2, K, GF], bf16, tag="e")
                    ex_t[pi] = e_pair
                for idx, ch in enumerate(group):
                    a = apool.tile([PART, K, GF], bf16, tag="a")
                    nc.sync.dma_start(
                        out=a[:].rearrange("p k q -> p (k q)"), in_=acts[ch]
                    )
                    a_t[idx] = a
                    nc.scalar.activation(
                        out=ex_t[idx // 2][:, idx % 2, 0],
                        in_=a[:],
                        func=mybir.ActivationFunctionType.Exp,
                    )
                nfc = gn - na  # leading C-chunks
                # C-chunks (head): XE = x * E on DVE (bf16 2x)
                for idx in range(nfc):
                    nc.vector.tensor_tensor(
                        ex_t[idx // 2][:, idx % 2, 1],
                        a_t[idx][:],
                        ex_t[idx // 2][:, idx % 2, 0],
                        mybir.AluOpType.mult,
                    )
                # A-chunks (tail): SY = silu(x - m) on ACT
                for idx in range(nfc, gn):
                    nc.scalar.activation(
                        out=ex_t[idx // 2][:, idx % 2, 1],
                        in_=a_t[idx][:],
                        func=mybir.ActivationFunctionType.Silu,
                        bias=mvec[:],
                    )

                # combined tree over k planes of chunk PAIRS (packed -> 2x):
                # zs[:, idx, 0, :] = Z, zs[:, idx, 1, :] = SY/XE sum
                zs = zpool.tile([PART, PH, 2, GF], bf16, tag="zs")
                for pi in range(gn // 2):
                    e = ex_t[pi]
                    t4 = tree.tile([PART, 2, 2, 4, GF], bf16, tag="t4")
                    nc.vector.tensor_add(
                        t4[:], e[:, :, :, 0:4, :], e[:, :, :, 4:8, :]
                    )
                    p2 = tree.tile([PART, 2, 2, 2, GF], bf16, tag="p2")
                    nc.vector.tensor_add(
                        p2[:], t4[:, :, :, 0:2, :], t4[:, :, :, 2:4, :]
                    )
                    q2 = tree.tile([PART, 2, 2, 2, GF], bf16, tag="q2")
                    nc.vector.tensor_add(q2[:], p2[:], e[:, :, :, 8:10, :])
                    nc.vector.tensor_add(
                        zs[:, 2 * pi : 2 * pi + 2],
                        q2[:, :, :, 0, :],
                        q2[:, :, :, 1, :],
                    )

                # batched smalls, issued per HALF-group (4 chunks) so the
                # DVE->ACT->DVE ping-pong (tree -> ln/rz -> meanx) starts as
                # soon as the first two pairs' trees land
                lz = lzpool.tile([PART, PH, PX_PER_PART, EW], bf16, tag="lz")
                rzt = rzpool.tile([PART, PH, GF], bf16, tag="rz")
                mx = mxpool.tile([PART, PH, GF], bf16, tag="mx")
                mk = mkpool.tile([PART, PH, PX_PER_PART, C], bf16, tag="mk")
                for h0 in range(0, gn, 4):
                    h1 = min(h0 + 4, gn)
                    nc.scalar.activation(
                        out=lz[:, h0:h1, :, 0:G],
                        in_=zs[:, h0:h1, 0, :].rearrange(
                            "p h (j g) -> p h j g", g=G
                        ),
                        func=mybir.ActivationFunctionType.Ln,
                    )
                    nc.vector.memset(lz[:, h0:h1, :, G : G + 1], 1.0)
                    # m-indicator col: 0 for C-chunks (idx < nfc), m for A
                    cpart0, cpart1 = h0, min(h1, nfc)
                    if cpart0 < cpart1:
                        nc.vector.memset(
                            lz[:, cpart0:cpart1, :, G + 1 : G + 2], 0.0
                        )
                    apart0, apart1 = max(h0, nfc), h1
                    if apart0 < apart1:
                        nc.vector.memset(
                            lz[:, apart0:apart1, :, G + 1 : G + 2], MSHIFT
                        )

                    # rz = 1/Z = exp(-logZ) on ACT (stays in exp/ln table set)
                    nc.scalar.activation(
                        out=rzt[:, h0:h1].rearrange("p h (j g) -> p h j g", g=G),
                        in_=lz[:, h0:h1, :, 0:G],
                        func=mybir.ActivationFunctionType.Exp,
                        scale=-1.0,
                    )
                    nc.vector.tensor_tensor(
                        mx[:, h0:h1],
                        zs[:, h0:h1, 1, :],
                        rzt[:, h0:h1],
                        mybir.AluOpType.mult,
                    )

                    lab_ap = lab_sb[:, g0 + h0 : g0 + h1, :]
                    lab_b = bass.AP(
                        tensor=lab_ap.tensor,
                        offset=lab_ap.offset,
                        ap=[lab_ap.ap[0], lab_ap.ap[1], lab_ap.ap[2], [0, C]],
                    )
                    iota_ap = iota_ps[:]
                    iota_b = bass.AP(
                        tensor=iota_ap.tensor,
                        offset=iota_ap.offset,
                        ap=[
                            iota_ap.ap[0],
                            [0, h1 - h0],
                            [0, PX_PER_PART],
                            iota_ap.ap[1],
                        ],
                    )
                    nc.vector.tensor_tensor(
                        mk[:, h0:h1], lab_b, iota_b, mybir.AluOpType.is_equal
                    )

                for idx, ch in enumerate(group):
                    nc.tensor.matmul(
                        out=stats_ps[:],
                        lhsT=mk[:, idx].rearrange("p j c -> p (j c)"),
                        rhs=lz[:, idx].rearrange("p j e -> p (j e)"),
                        start=(ch == 0),
                        stop=(ch == NCHUNK - 1),
                        skip_group_check=True,
                    )
                    if idx >= nfc:
                        nc.tensor.matmul(
                            out=stats2a_ps[:],
                            lhsT=mk[:, idx].rearrange("p j c -> p (j c)"),
                            rhs=mx[:, idx],
                            start=(ch == first_a),
                            stop=(ch == last_a),
                            skip_group_check=True,
                        )
                    else:
                        nc.tensor.matmul(
                            out=stats2b_ps[:],
                            lhsT=mk[:, idx].rearrange("p j c -> p (j c)"),
                            rhs=mx[:, idx],
                            start=(ch == first_c),
                            stop=(ch == last_c),
                            skip_group_check=True,
                        )

            stats_sb = singles.tile([PX_PER_PART * C, PX_PER_PART * EW], f32)
            nc.vector.tensor_copy(out=stats_sb[:], in_=stats_ps[:])
            nc.sync.dma_start(out=stats_out, in_=stats_sb[:])
            stats2a_sb = singles.tile([PX_PER_PART * C, PX_PER_PART * G], f32)
            nc.vector.tensor_copy(out=stats2a_sb[:], in_=stats2a_ps[:])
            nc.sync.dma_start(out=stats2a_out, in_=stats2a_sb[:])
            stats2b_sb = singles.tile([PX_PER_PART * C, PX_PER_PART * G], f32)
            nc.vector.tensor_copy(out=stats2b_sb[:], in_=stats2b_ps[:])
            nc.sync.dma_start(out=stats2b_out, in_=stats2b_sb[:])

    nc.compile()
    _CACHE["nc"] = nc
    return nc


def _prep_inputs(prototype_activations, target_labels, proto_idx):
    acts = np.asarray(prototype_activations, dtype=np.float32)
    labels = np.asarray(target_labels)
    pidx = np.asarray(proto_idx)

    expected = np.arange(S * C * K, dtype=np.int64).reshape(S, C, K)
    if not np.array_equal(pidx.astype(np.int64), expected):
        # general (slow) fallback: permute proto columns on host
        acts = np.ascontiguousarray(acts[..., pidx.reshape(-1)])

    labels_f = labels.astype(np.float32)
    consts = np.concatenate(
        [np.arange(1, C + 1, dtype=np.float32), np.ones(1, dtype=np.float32)]
    )

    in_maps = []
    for b in range(B):
        lab_b = np.ascontiguousarray(
            labels_f[b]
            .reshape(NCHUNK, PART, PX_PER_PART)
            .transpose(1, 0, 2)
            .reshape(PART, NCHUNK * PX_PER_PART)
        )
        # k-major per chunk: (j, g, k) -> (k, j, g) so every on-chip operand
        # is contiguous/packed (DVE 2x on all tree levels); upload in bf16
        # (RNE cast) to halve HBM traffic -- per-element ~0.2% noise is
        # random across 2M pixel-groups and cancels in the final mean.
        import ml_dtypes

        acts_b = (
            acts[b]
            .reshape(NCHUNK, PART, PX_PER_PART * G, K)
            .transpose(0, 1, 3, 2)
            .reshape(NCHUNK, PART, FREE)
        )
        in_maps.append(
            {
                "acts": np.ascontiguousarray(acts_b).astype(ml_dtypes.bfloat16),
                "labels": lab_b,
                "consts": consts,
            }
        )
    return in_maps


def _combine(stats_list):
    """stats_list: per-core (st1 [64, 272], st2a [64, 256], st2b [64, 256])."""
    num = np.zeros((B, S, C), dtype=np.float32)
    cnt = np.zeros((B, C), dtype=np.float32)
    jj = np.arange(PX_PER_PART)
    em = np.float32(np.exp(MSHIFT))
    for b, (st1, st2a, st2b) in enumerate(stats_list):
        st1 = st1.reshape(PX_PER_PART, C, PX_PER_PART, EW)  # [j, c, j', e]
        d1 = st1[jj, :, jj, :].sum(axis=0)  # [c, 34]
        d2a = (
            st2a.reshape(PX_PER_PART, C, PX_PER_PART, G)[jj, :, jj, :].sum(axis=0)
        )  # [c, 32] of sum SS/Z over A-chunk pixels
        d2b = (
            st2b.reshape(PX_PER_PART, C, PX_PER_PART, G)[jj, :, jj, :].sum(axis=0)
        )  # [c, 32] of sum XE/Z over B-chunk pixels
        cntc = d1[:, G]
        mcorr = d1[:, G + 1]  # = MSHIFT * (A-chunk count per class)
        # ent = logZ - meanx; meanx_A = e^m*(SS/Z) + m, meanx_B = XE/Z
        ent_cols = d1[:, :G] - em * d2a - d2b - mcorr[:, None]
        ent_cols = ent_cols.reshape(C, S, C)
        num[b] = ent_cols[np.arange(C), :, np.arange(C)].T  # [s, c]
        cnt[b] = cntc
    num /= np.float32(np.log(K))
    present = cnt > 0
    mean_ent = num / np.maximum(cnt, 1.0)[:, None, :]
    n_entries = np.float32(present.sum() * S)
    total = np.float32((mean_ent * present[:, None, :]).sum(dtype=np.float64))
    if n_entries > 0:
        out = np.float32(total / max(n_entries, np.float32(1.0)))
    else:
        out = np.float32(0.0)
    return out


def kernel(prototype_activations, target_labels, proto_idx, _trace=False, _tmpdir=None):
    nc = _build()
    in_maps = _prep_inputs(prototype_activations, target_labels, proto_idx)
    res = run_bass_kernel_spmd(
        nc, in_maps, list(range(NCORES)), trace=_trace, tmpdir=_tmpdir
    )
    stats_list = [
        (
            res.results[i]["stats"],
            res.results[i]["stats2a"],
            res.results[i]["stats2b"],
        )
        for i in range(NCORES)
    ]
    out = _combine(stats_list)
    if _trace:
        return out, res
    return out

